# revision 41
# baseline (speedup 1.0000x reference)
"""Complex Conv1D (VALID, stride 1) on Trainium2 — Bass/Tile, 8-core data-parallel.

Problem (hardcoded shapes):
  x_real/x_imag: [32, 4096, 64] f32, kernel_real/imag: [9, 64, 64] f32,
  bias_real/imag: [64] f32  ->  out [32, 4088, 64, 2] f32
  out_real = conv(xr, wr) - conv(xi, wi) + br
  out_imag = conv(xr, wi) + conv(xi, wr) + bi

Mapping: complex multiply as its 2x2 real block-matrix form so each tap is ONE
full 128-contract matmul:
  X_b [128, L]   rows 0:64 = xr[b].T (channels on partitions), 64:128 = xi[b].T
  W[k] [128,128] = [[wr[k], wi[k]], [-wi[k], wr[k]]]
  psum[128, T] += W[k].T @ X_b[:, l0+k : l0+k+T]   for k = 0..8
  psum rows 0:64 = real output (filters), rows 64:128 = imag output.
Batch is sharded 4-per-core across 8 cores; weights replicated. The kernel
emits the output transposed as [b, 128, L_out] bf16; the host restores
[B, L_out, F, 2] f32.

Perf notes (core-0 NTFF profile):
  - bf16 operands stream the PE at 1 cycle/row (216 ns/MM at N=512); the
    288-MM stream is the ~61.4 us floor for this algorithm. fp32r measures
    ~232 ns/MM; bf16 also halves DMA bytes and enables fast weight load.
  - 30 fine-grained warmup matmuls on a memset scratch tile keep the PE
    busy through the DMA head so the HAM clock gate is (usually) at 8/8
    when the real stream starts.
  - DMA queues take ~2-2.5 us from first doorbell until packets flow, so
    the first x tile is the FIRST sync-queue DMA, followed by the weights
    in three 3-tap chunks (tap-0 LDWEIGHTS can start ~1 us earlier).
  - Keep late-kernel DMAs on the sync queue only: a store issued on a
    cold scalar/gpsimd queue at the end pays the full spin-up, and
    gpsimd's teardown DRAIN is ~5 us when it has issued DMAs.
  - PSUM evacuation via scalar ACTIVATE (680 ns; vector tensor_scalar is
    751 ns and PSUM-read-bound, not faster).
"""

import numpy as np

import concourse.bacc as bacc
import concourse.bass as bass
import concourse.mybir as mybir
from concourse.tile import TileContext
from concourse.bass_utils import run_bass_kernel_spmd

B, L, CIN, KT, F = 32, 4096, 64, 9, 64
LOUT = L - KT + 1  # 4088
NCORES = 8
BPC = B // NCORES  # batches per core
TL = 512  # output-tile width (one PSUM bank of fp32)
NLT = (LOUT + TL - 1) // TL  # 8

MM_DT_NAME = "bfloat16"
OUT_DT_NAME = "bfloat16"
WARMUP = 34


def _build_nc(
    mm_dt,
    w_dt=None,
    out_dt=None,
    xbufs=6,
    obufs=3,
    psbufs=4,
    warmup=WARMUP,
    evac="act",
    batch_x=False,
    ostore2=True,
    xpair=False,
    lastsplit=True,
):
    nc = bacc.Bacc("TRN2", target_bir_lowering=False, debug=False, num_devices=NCORES)
    if w_dt is None:
        w_dt = mm_dt
    if out_dt is None:
        out_dt = getattr(mybir.dt, OUT_DT_NAME)

    f32 = mybir.dt.float32

    x_d = nc.dram_tensor("x", [BPC, 128, L], mm_dt, kind="ExternalInput")
    w_d = nc.dram_tensor("w", [128, KT * 128], w_dt, kind="ExternalInput")
    bias_d = nc.dram_tensor("bias", [128, 1], f32, kind="ExternalInput")
    out_d = nc.dram_tensor("out", [BPC, 128, LOUT], out_dt, kind="ExternalOutput")

    ident = mybir.ActivationFunctionType.Identity

    with TileContext(nc) as tc:
        with (
            tc.tile_pool(name="wpool", bufs=1) as wpool,
            tc.tile_pool(name="xpool", bufs=xbufs) as xpool,
            tc.tile_pool(name="xbpool", bufs=2) as xbpool,
            tc.tile_pool(name="opool", bufs=obufs) as opool,
            tc.tile_pool(name="pspool", bufs=psbufs, space="PSUM") as pspool,
        ):
            # Warmup scratch memset is gpsimd's FIRST instruction so the PE
            # warmup stream starts right after the framework preamble. All
            # x-tile loads ride the gpsimd queue; the weights ride sync in
            # three 3-tap chunks (tap-0 weights land ~1us earlier than one
            # big DMA would), so x0 and w transfer in parallel. Bias rides
            # the otherwise-idle scalar queue. Output stores share sync.
            sc = None
            if warmup:
                sc = wpool.tile([128, 128], mm_dt)
                nc.gpsimd.memset(sc[:], 0.0)
            TL2 = 2 * TL
            xw = (TL2 if xpair else TL) + KT - 1
            xt0 = xpool.tile([128, xw], mm_dt, tag="xt")
            nc.sync.dma_start(xt0[:], x_d[0, :, 0:xw])
            wt = wpool.tile([128, KT * 128], w_dt)
            for c in range(3):
                nc.sync.dma_start(
                    wt[:, c * 384 : (c + 1) * 384], w_d[:, c * 384 : (c + 1) * 384]
                )
            bias_t = wpool.tile([128, 1], f32)
            nc.scalar.dma_start(bias_t[:], bias_d[:])

            if warmup:
                # Keep the PE busy >=3.4us (HAM un-throttle window) while
                # the first DMAs land. Fine-grained N=128 matmuls (~107ns
                # cold) so the real stream isn't delayed by a coarse tail.
                wps = pspool.tile([128, 128], f32, tag="wps", bufs=1)
                for _ in range(warmup):
                    nc.tensor.matmul(
                        wps[:], sc[:], sc[:],
                        start=True, stop=True, skip_group_check=True,
                    )

            # Batches 1..BPC-1 load as one 1MB DMA each on the gpsimd
            # queue (fewer, bigger packets; decoupled from the per-tile
            # cadence). Batch 0 keeps per-tile loads for a fast start.
            xbs = {}
            if batch_x:
                for bb in range(1, BPC):
                    xb = xbpool.tile([128, L], mm_dt, tag="xb")
                    nc.gpsimd.dma_start(xb[:], x_d[bb, :, :])
                    xbs[bb] = xb

            for b in range(BPC):
                for j in range(NLT):
                    l0 = j * TL
                    t = min(TL, LOUT - l0)
                    w_in = min(L, l0 + t + KT - 1) - l0
                    off = 0
                    if xpair:
                        # One [128, 1032] load serves two adjacent tiles:
                        # halves load issues, doubles DMA packet size.
                        if j % 2 == 0:
                            P0 = (j // 2) * TL2
                            w_in2 = min(L, P0 + TL2 + KT - 1) - P0
                            if b == 0 and j == 0:
                                xt_pair = xt0
                            else:
                                xt_pair = xpool.tile([128, xw], mm_dt, tag="xt")
                                nc.sync.dma_start(
                                    xt_pair[:, :w_in2], x_d[b, :, P0 : P0 + w_in2]
                                )
                        xt = xt_pair
                        off = (j % 2) * TL
                    elif b == 0 and j == 0:
                        xt = xt0
                    elif batch_x and b >= 1:
                        xt = xbs[b]
                        off = l0
                    else:
                        xt = xpool.tile([128, xw], mm_dt, tag="xt")
                        nc.sync.dma_start(xt[:, :w_in], x_d[b, :, l0 : l0 + w_in])
                    ps = pspool.tile([128, TL], f32, tag="ps")
                    for k in range(KT):
                        nc.tensor.matmul(
                            ps[:, :t],
                            wt[:, k * 128 : (k + 1) * 128],
                            xt[:, off + k : off + k + t],
                            start=(k == 0),
                            stop=(k == KT - 1),
                        )
                    if ostore2:
                        # Pair two adjacent tiles into one [128, 2*TL] SBUF
                        # buffer and store them with a single DMA: halves
                        # the sync-queue issue count and store packets, and
                        # measures both faster and far more consistent than
                        # per-tile stores (80.8-81.0us vs 80-85us spread).
                        last_pair = lastsplit and b == BPC - 1 and j >= NLT - 2
                        if j % 2 == 0:
                            ot2 = opool.tile([128, 2 * TL], out_dt, tag="ot")
                            nc.scalar.activation(
                                ot2[:, :t], ps[:, :t], ident, bias=bias_t[:]
                            )
                            if last_pair:
                                # Final pair: store each half as soon as it
                                # is ready so the exposed tail transfer is
                                # half as long.
                                nc.sync.dma_start(
                                    out_d[b, :, l0 : l0 + t], ot2[:, :t]
                                )
                        else:
                            nc.scalar.activation(
                                ot2[:, TL : TL + t], ps[:, :t], ident, bias=bias_t[:]
                            )
                            if last_pair:
                                nc.sync.dma_start(
                                    out_d[b, :, l0 : l0 + t], ot2[:, TL : TL + t]
                                )
                            else:
                                nc.sync.dma_start(
                                    out_d[b, :, l0 - TL : l0 + t], ot2[:, : TL + t]
                                )
                        continue
                    ot = opool.tile([128, TL], out_dt, tag="ot")
                    if evac == "vec":
                        nc.vector.tensor_scalar_add(ot[:, :t], ps[:, :t], bias_t[:])
                    else:
                        nc.scalar.activation(
                            ot[:, :t], ps[:, :t], ident, bias=bias_t[:]
                        )
                    nc.sync.dma_start(out_d[b, :, l0 : l0 + t], ot[:, :t])

    nc.compile()
    return nc


def _pack(x_real, x_imag, kernel_real, kernel_imag, bias_real, bias_imag, np_dt,
          w_np_dt=None):
    if w_np_dt is None:
        w_np_dt = np_dt
    X = np.empty((B, 128, L), np_dt)
    X[:, :CIN] = x_real.transpose(0, 2, 1)
    X[:, CIN:] = x_imag.transpose(0, 2, 1)
    Wk = np.empty((KT, 128, 128), np.float32)
    Wk[:, :CIN, :F] = kernel_real
    Wk[:, :CIN, F:] = kernel_imag
    Wk[:, CIN:, :F] = -kernel_imag
    Wk[:, CIN:, F:] = kernel_real
    W2 = Wk.transpose(1, 0, 2).reshape(128, KT * 128).astype(w_np_dt)
    bias2 = (
        np.concatenate([bias_real, bias_imag]).reshape(128, 1).astype(np.float32)
    )
    return X, np.ascontiguousarray(W2), bias2


def _parse_dt(name):
    name = name or MM_DT_NAME
    if "," in name:
        xn, wn = name.split(",")
    else:
        xn = wn = name
    return getattr(mybir.dt, xn), getattr(mybir.dt, wn)


_NC_CACHE = {}


def _prepare(inputs, mm_dt_name=None, build_kw=None):
    mm_dt, w_dt = _parse_dt(mm_dt_name)
    np_dt = mybir.dt.np(mm_dt)
    w_np_dt = mybir.dt.np(w_dt)
    args = {
        k: np.asarray(inputs[k], np.float32)
        for k in (
            "x_real", "x_imag", "kernel_real", "kernel_imag", "bias_real", "bias_imag",
        )
    }
    X, W2, bias2 = _pack(np_dt=np_dt, w_np_dt=w_np_dt, **args)

    # The compiled Bass module is input-independent; reuse it across calls.
    key = (mm_dt_name, repr(sorted((build_kw or {}).items())))
    nc = _NC_CACHE.get(key)
    if nc is None:
        nc = _build_nc(mm_dt, w_dt=w_dt, **(build_kw or {}))
        _NC_CACHE[key] = nc
    in_maps = [
        {
            "x": np.ascontiguousarray(X[i * BPC : (i + 1) * BPC]),
            "w": W2,
            "bias": bias2,
        }
        for i in range(NCORES)
    ]
    return nc, in_maps


def _gather(results):
    O = np.concatenate([r["out"] for r in results], axis=0)  # [32, 128, 4088]
    O = O.astype(np.float32).reshape(B, 2, F, LOUT).transpose(0, 3, 2, 1)
    return np.ascontiguousarray(O, dtype=np.float32)


def _run(inputs, trace=False, mm_dt_name=None, build_kw=None):
    nc, in_maps = _prepare(inputs, mm_dt_name, build_kw=build_kw)
    res = run_bass_kernel_spmd(nc, in_maps, core_ids=list(range(NCORES)), trace=trace)
    return _gather(res.results), res


def kernel(**inputs) -> np.ndarray:
    out, _ = _run(inputs, trace=False)
    return out
